# revision 2
# baseline (speedup 1.0000x reference)
"""EnhancedChannelFilter Trainium2 kernel, v2c.

Structure vs the original v1 kernel:
  1. Host computes the packet-loss mask, pre-applies it to x (xm = x*mask),
     computes the SE chain (y = mean(xm), scores, mc) exactly, and folds the
     per-image output scale mc into per-image rec2 stationaries. The device
     runs ONLY det -> sigmoid -> zh -> rec1 -> relu -> rec2.
  2. Dead-pixel compaction: at rate=50 ~34% of pixels have ALL 16 channel
     groups masked -> their z column is exactly 0 -> output column exactly 0.
     Host gathers live pixels only; device computes ~2080/3136 columns.
     Images are sorted by live-count and assigned (core, slot) so each slot
     has a tight compile-time padded size.
  3. det runs as 1-term fp8 DoubleRow (xm8 shipped from host): measured on
     HW a DR matmul covering K=256 costs ~the same as ONE bf16 K=128 matmul,
     so 1-term fp8 is the only real 2x; residual-pair ("2-term") DR schemes
     cost the same as bf16 and were dropped. det's fp8 error is squashed by
     the sigmoid. rec1/rec2 stay bf16 (accuracy), zh is bf16 (fast 2x DVE).
  4. rec2's two output groups (128 + L2p live channels) accumulate into one
     2-bank psum tile, evicted by a single ACT copy into SBUF bf16.
  5. Engine budget per 440-px tile (HW-measured): PE 2.52us, ACT 2.1us,
     DVE 1.7us. Software pipeline: rec1B/relu lag one tile behind det
     (covers det->sigmoid->zh latency); rec2/evict lag 2 more; slot0's x is
     wrap-prefetched during slot3 of the previous repeat-loop iteration.
"""

import math

import numpy as np
import ml_dtypes

B, C, H, W = 32, 256, 56, 56
HW = H * W              # 3136
NCORES = 8
BC = B // NCORES        # images per core (slots)
NSLOT = BC

_CACHE: dict = {}
_CFG: dict = {}


def _split_multiwaits(nc, mybir):
    """This walrus build enforces 1 sync wait per instruction (2 for
    EventSemaphore); the Tile framework attaches several to its exit drain."""
    n = 0
    for bb in nc.m.functions[0].blocks:
        lst = bb.instructions
        for inst in list(lst):
            si = inst.sync_info
            if si is None or not si.on_wait:
                continue
            cap = 2 if isinstance(inst, mybir.InstEventSemaphore) else 1
            waits = list(si.on_wait)
            if len(waits) <= cap:
                continue
            eng = nc.engines[inst.engine]
            extra = []
            for wt in waits[:-cap]:
                nop = eng.nop(nofuse=True).ins
                nop.sync_info = mybir.SyncInfo(on_wait=[wt], on_update=[])
                nc.cur_bb.bb.instructions.remove(nop)
                extra.append(nop)
            si.on_wait = waits[-cap:]
            idx = lst.index(inst)
            lst[idx:idx] = extra
            n += 1
    return n


def _tiles_of(S):
    """Split padded slot size S into <=512-col tiles (multiples of 8)."""
    nt = max(1, math.ceil(S / 512))
    t = math.ceil(S / nt / 8) * 8
    tiles = []
    rem = S
    while rem > 0:
        w = min(t, rem)
        tiles.append(w)
        rem -= w
    return tiles


def _build(debug=False, repeat=0):
    import concourse.bass as bass
    import concourse.tile as tile
    import concourse.mybir as mybir

    L1 = _CFG["L1"]
    L2p = _CFG["L2p"]
    S = _CFG["S"]               # per-slot padded sizes, len NSLOT
    b1t = _CFG.get("b1t", False)  # rec1B as 1-term fp8 DR (faster, more err)
    STOT = sum(S)
    OFF = [sum(S[:i]) for i in range(NSLOT)]
    LP = L1 + L2p

    f32 = mybir.dt.float32
    bf16 = mybir.dt.bfloat16
    fp8 = mybir.dt.float8e4
    DR = mybir.MatmulPerfMode.DoubleRow
    MULT = mybir.AluOpType.mult
    SIGMOID = mybir.ActivationFunctionType.Sigmoid
    RELU = mybir.ActivationFunctionType.Relu
    COPY = mybir.ActivationFunctionType.Copy
    BAL = "v1"

    nc = bass.Bass("TRN2", target_bir_lowering=False, debug=False)

    x_d = nc.dram_tensor("xall", [128, 2, STOT], bf16, kind="ExternalInput").ap()
    x8_d = nc.dram_tensor("x8all", [128, 2, STOT], fp8, kind="ExternalInput").ap()
    # rec1 bf16 stationaries: [128, 2(slot), 4(r1a mh0/1, r1b mh0/1), 128]
    wg_d = nc.dram_tensor("wg", [128, 2, 4, 128], bf16, kind="ExternalInput").ap()
    # det fp8 DR stationaries D8 (+ rec1B B8 if b1t): [128, 2(slot), 4, 128]
    wd8_d = nc.dram_tensor("wd8", [128, 2, 4, 128], fp8, kind="ExternalInput").ap()
    # rec2 stationaries with mc folded, per slot; group2 zero-padded to 128
    # cols so its matmul initializes the full psum bank.
    LPW = L1 + 128 if L2p else L1
    w2_d = nc.dram_tensor("w2", [128, 2, NSLOT, LPW], bf16, kind="ExternalInput").ap()
    out_d = nc.dram_tensor("out2", [LP, STOT], bf16, kind="ExternalOutput").ap()
    if debug:
        dsg_d = nc.dram_tensor("dsg", [128, 2, STOT], bf16, kind="ExternalOutput").ap()
        dr1_d = nc.dram_tensor("dr1", [128, 2, STOT], bf16, kind="ExternalOutput").ap()

    zhdt = fp8 if b1t else bf16

    with tile.TileContext(nc) as tc:
        with (
            tc.tile_pool(name="consts", bufs=1) as cpool,
            tc.tile_pool(name="xin", bufs=1) as xpool,
            tc.tile_pool(name="sg", bufs=3) as sgpool,
            tc.tile_pool(name="zh", bufs=3) as zhpool,
            tc.tile_pool(name="r1", bufs=4) as r1pool,
            tc.tile_pool(name="osb", bufs=3) as opool,
            tc.tile_pool(name="dp", bufs=1, space="PSUM") as dppool,
            tc.tile_pool(name="r1p", bufs=2, space="PSUM") as r1ppool,
            tc.tile_pool(name="r2p", bufs=1, space="PSUM") as r2ppool,
        ):
            wg = cpool.tile([128, 2, 4, 128], bf16, name="wg", tag="wg")
            wd8 = cpool.tile([128, 2, 4, 128], fp8, name="wd8", tag="wd8")
            w2sb = cpool.tile([128, 2, NSLOT, LPW], bf16, name="w2sb", tag="w2sb")
            xall = xpool.tile([128, 2, STOT], bf16, name="xall", tag="xall")
            x8all = xpool.tile([128, 2, STOT], fp8, name="x8all", tag="x8all")
            nc.sync.dma_start(wd8[:], wd8_d[:])
            nc.sync.dma_start(w2sb[:], w2_d[:])
            if repeat:
                nc.sync.dma_start(wg[:], wg_d[:])

            wdet8 = [wd8[:, :, mh, :] for mh in range(2)]
            wr1a = [wg[:, :, 0 + mh, :] for mh in range(2)]
            wr1b = [wg[:, :, 2 + mh, :] for mh in range(2)]
            wr1b8 = [wd8[:, :, 2 + mh, :] for mh in range(2)]

            def in_dma(b, force=False):
                o0, o1 = OFF[b], OFF[b] + S[b]
                om = (o0 + o1) // 2
                nc.sync.dma_start(xall[:, :, o0:om], x_d[:, :, o0:om])
                if b == 0 and not repeat:
                    nc.sync.dma_start(wg[:], wg_d[:])
                nc.sync.dma_start(xall[:, :, om:o1], x_d[:, :, om:o1])
                nc.sync.dma_start(x8all[:, :, o0:o1], x8_d[:, :, o0:o1])

            def p_front(b, j, n0, NT):
                """det + sigmoid + zh + rec1-xm; returns rec1-zh+relu
                closure."""
                xmn = xall[:, :, n0:n0 + NT]
                x8n = x8all[:, :, n0:n0 + NT]

                dp = dppool.tile([128, 1024], f32, name=f"dp_b{b}j{j}", tag="dp")
                for mh in range(2):
                    nc.tensor.matmul(
                        dp[:, mh * 512:mh * 512 + NT],
                        wdet8[mh], x8n,
                        start=True, stop=True, perf_mode=DR,
                    )
                sg = sgpool.tile([128, 2, 512], bf16, name=f"sg_b{b}j{j}", tag="sg")
                dpv = dp.rearrange("p (m w) -> p m w", w=512)
                nc.scalar.activation(sg[:, :, 0:NT], dpv[:, :, 0:NT], SIGMOID)
                if debug:
                    nc.sync.dma_start(dsg_d[:, :, n0:n0 + NT], sg[:, :, 0:NT])
                zh = zhpool.tile([128, 2, 512], zhdt, name=f"zh_b{b}j{j}", tag="zh")
                if BAL == "v2":
                    nc.gpsimd.tensor_tensor(zh[:, :, 0:NT], sg[:, :, 0:NT], xmn,
                                            MULT)
                else:
                    nc.vector.tensor_tensor(zh[:, :, 0:NT], sg[:, :, 0:NT], xmn,
                                            MULT)

                r1p = r1ppool.tile([128, 1024], f32, name=f"r1p_b{b}j{j}", tag="r1p")
                pvs = [r1p[:, mh * 512:mh * 512 + NT] for mh in range(2)]
                for mh in range(2):
                    for s in range(2):
                        nc.tensor.matmul(pvs[mh], wr1a[mh][:, s, :], xmn[:, s, :],
                                         start=(s == 0), stop=False)

                def fin():
                    for mh in range(2):
                        if b1t:
                            nc.tensor.matmul(pvs[mh], wr1b8[mh], zh[:, :, 0:NT],
                                             start=False, stop=True,
                                             perf_mode=DR)
                        else:
                            for s in range(2):
                                nc.tensor.matmul(pvs[mh], wr1b[mh][:, s, :],
                                                 zh[:, s, 0:NT],
                                                 start=False, stop=(s == 1))
                    r1 = r1pool.tile([128, 2, 512], bf16, name=f"r1_b{b}j{j}",
                                     tag="r1")
                    r1pv = r1p.rearrange("p (m w) -> p m w", w=512)
                    if BAL in ("v1", "v2"):
                        nc.scalar.activation(r1[:, :, 0:NT], r1pv[:, :, 0:NT],
                                             RELU)
                    else:
                        nc.vector.tensor_scalar_max(r1[:, :, 0:NT],
                                                    r1pv[:, :, 0:NT], 0.0)
                    if debug:
                        nc.sync.dma_start(dr1_d[:, :, n0:n0 + NT], r1[:, :, 0:NT])
                    return r1

                return fin

            def p_back(b, j, n0, NT, r1, half, last):
                """rec2 into one 2-bank psum tile; single ACT evict; DMA per
                output pair (half/last as computed in pairpos)."""
                r2 = r2ppool.tile([128, 1024], f32, name=f"r2_b{b}j{j}",
                                  tag="r2p")
                for k in range(2):
                    nc.tensor.matmul(r2[:, 0:NT], w2sb[:, k, b, 0:L1],
                                     r1[:, k, 0:NT],
                                     start=(k == 0), stop=(k == 1))
                if L2p:
                    for k in range(2):
                        nc.tensor.matmul(r2[:, 512:512 + NT],
                                         w2sb[:, k, b, L1:LPW],
                                         r1[:, k, 0:NT],
                                         start=(k == 0), stop=(k == 1))
                if half == 0:
                    ot[0] = opool.tile([128, 2, 2, 512], bf16,
                                       name=f"oa_b{b}j{j}", tag="oa")
                oa = ot[0]
                r2v = r2.rearrange("p (g w) -> p g w", w=512)
                if BAL in ("v1", "v2"):
                    nc.vector.tensor_copy(oa[:, half, :, 0:NT], r2v[:, :, 0:NT])
                else:
                    nc.scalar.activation(oa[:, half, :, 0:NT], r2v[:, :, 0:NT],
                                         COPY)
                if last:
                    n0p = n0 - half * NT
                    w = half + 1
                    nc.sync.dma_start(out_d[0:L1, n0p:n0p + w * NT],
                                      oa[0:L1, 0:w, 0, 0:NT])
                    if L2p:
                        nc.sync.dma_start(out_d[L1:LP, n0p:n0p + w * NT],
                                          oa[0:L2p, 0:w, 1, 0:NT])

            ot = {}

            import contextlib as _ctxlib
            rep_cm = (tc.For_i(0, repeat, 1,
                               hint_engines=(mybir.EngineType.PE,
                                             mybir.EngineType.DVE,
                                             mybir.EngineType.Activation,
                                             mybir.EngineType.SP))
                      if repeat else _ctxlib.nullcontext())
            if repeat:
                in_dma(0, force=True)   # preamble fill; loop wraps thereafter
            with rep_cm:
                from collections import deque
                work = []
                for b in range(NSLOT):
                    n0 = OFF[b]
                    for j, NT in enumerate(_tiles_of(S[b])):
                        work.append((b, j, n0, NT))
                        n0 += NT
                # output pairing: tiles (0,1)/(2,3) of a slot share one out
                # tile + DMA; odd tails go solo.
                pairpos = []
                for b2 in range(NSLOT):
                    ts = _tiles_of(S[b2])
                    for j2 in range(len(ts)):
                        if j2 % 2 == 0 and j2 + 1 < len(ts) and \
                                ts[j2] == ts[j2 + 1]:
                            pairpos.append((0, False))
                        elif j2 % 2 == 1 and ts[j2] == ts[j2 - 1]:
                            pairpos.append((1, True))
                        else:
                            pairpos.append((0, True))
                if not repeat:
                    in_dma(0)
                pend = deque()
                pfin = None
                NW = len(work)
                for w_i, (b, j, n0, NT) in enumerate(work):
                    if j == 0 and (b + 1 < NSLOT or repeat):
                        in_dma((b + 1) % NSLOT)
                    # rec2/evict of tile t-2 go FIRST (the ready DVE evict
                    # must not queue behind zh(t) waiting on sigmoid(t));
                    # near the tail drain the stagger fully so the final
                    # evict->DMA chains start as early as possible.
                    depth = 2 if w_i < NW - 2 else 0
                    while len(pend) > depth:
                        wp = pend.popleft()
                        p_back(*wp[0:5], *pairpos[wp[5]])
                    fin = p_front(b, j, n0, NT)
                    if pfin is not None:
                        pend.append((pfin[0], pfin[1], pfin[2], pfin[3],
                                     pfin[4](), pfin[5]))
                    pfin = (b, j, n0, NT, fin, w_i)
                pend.append((pfin[0], pfin[1], pfin[2], pfin[3], pfin[4](),
                             pfin[5]))
                while pend:
                    wp = pend.popleft()
                    p_back(*wp[0:5], *pairpos[wp[5]])

    _split_multiwaits(nc, mybir)
    return nc


def _jax_perm_cpu(num_chunks: int) -> np.ndarray:
    import os
    import subprocess
    import sys
    import tempfile

    import jax

    sp = os.path.dirname(os.path.dirname(jax.__file__))
    code = (
        "import sys, numpy as np, jax\n"
        f"perm = np.asarray(jax.random.permutation(jax.random.key(1234), {num_chunks}))\n"
        "np.save(sys.argv[1], perm)\n"
    )
    with tempfile.TemporaryDirectory() as td:
        path = os.path.join(td, "perm.npy")
        env = dict(os.environ, JAX_PLATFORMS="cpu", PYTHONPATH=sp)
        env.pop("TRN_TERMINAL_POOL_IPS", None)
        subprocess.run([sys.executable, "-c", code, path], env=env, check=True)
        return np.load(path)


def _prep_in_maps(inputs, b1t=None):
    x = np.asarray(inputs["x"], dtype=np.float32).reshape(B, C, HW)
    rate = int(np.asarray(inputs["Packet_Loss_Rate"]))
    fc1 = np.asarray(inputs["fc1_w"], dtype=np.float32)
    fc2 = np.asarray(inputs["fc2_w"], dtype=np.float32)
    thr = float(np.asarray(inputs["threshold"], dtype=np.float32).reshape(-1)[0])
    detw = np.asarray(inputs["detect_w"], dtype=np.float32)
    r1w = np.asarray(inputs["rec1_w"], dtype=np.float32)
    r2w = np.asarray(inputs["rec2_w"], dtype=np.float32)
    aw = np.asarray(inputs["adapt_w"], dtype=np.float32)
    if b1t is None:
        b1t = _CFG.get("b1t", False)

    # ---- mask in channel-group space (group = c // 16), exact ----
    n = B * C * HW
    num_chunks = math.ceil(n * 4 / 1472)
    num_lossy = int(math.ceil(num_chunks * (rate / 100)))
    keep = np.ones((num_chunks,), np.float32)
    if num_lossy > 0:
        perm_c = _jax_perm_cpu(num_chunks)
        keep[perm_c[:num_lossy]] = 0.0
    bg = np.arange(B, dtype=np.int64)[:, None, None]
    qq = np.arange(16, dtype=np.int64)[None, :, None]
    pp = np.arange(HW, dtype=np.int64)[None, None, :]
    m16 = keep[((bg * HW + pp) * 16 + qq) // 23]      # [B, 16, HW] 0/1

    xm = x * np.repeat(m16, 16, axis=1)               # [B, C, HW]

    # ---- SE chain on host, exact ----
    y = xm.mean(axis=2)                               # [B, C]
    h1 = np.maximum(y @ fc1.T, 0.0)
    scores = 1.0 / (1.0 + np.exp(-(h1 @ fc2.T)))
    a = rate * aw[:, 0] - thr
    mc = np.maximum(scores + a[None, :], 0.0)         # [B, C]

    live_ch = np.where((mc > 0).any(axis=0))[0]
    live_ch = live_ch[np.argsort(-mc[:, live_ch].mean(axis=0))]
    L = len(live_ch)
    L1 = min(L, 128)
    L2 = L - L1
    L2p = ((L2 + 15) // 16) * 16 if L2 else 0
    LP = L1 + L2p

    # ---- dead-pixel compaction: per-image live pixel lists ----
    livepx = m16.any(axis=1)                          # [B, HW]
    NL = livepx.sum(axis=1)
    order = np.argsort(-NL, kind="stable")            # images by NL desc
    slot_of = np.empty(B, np.int64)
    core_of = np.empty(B, np.int64)
    for r, bimg in enumerate(order):
        core_of[bimg] = r % NCORES
        slot_of[bimg] = r // NCORES
    S = []
    for s in range(NSLOT):
        mx = max(int(NL[bimg]) for bimg in range(B) if slot_of[bimg] == s)
        S.append(((mx + 7) // 8) * 8)
    OFF = [sum(S[:i]) for i in range(NSLOT)]
    STOT = sum(S)

    _CFG.update(L1=L1, L2p=L2p, S=tuple(S), b1t=b1t)

    # ---- per-core inputs ----
    xm16 = xm.reshape(B, 128, 2, HW).astype(ml_dtypes.bfloat16)
    xm8 = xm.reshape(B, 128, 2, HW).astype(ml_dtypes.float8_e4m3)

    pp128 = np.arange(128)
    wg = np.zeros((128, 2, 4, 128), np.float32)
    for mh in range(2):
        for s in range(2):
            wg[:, s, 0 + mh, :] = r1w[mh * 128 + pp128][:, 2 * pp128 + s].T
            wg[:, s, 2 + mh, :] = r1w[mh * 128 + pp128][:, 256 + 2 * pp128 + s].T
    wg16 = wg.astype(ml_dtypes.bfloat16)

    wd = np.zeros((128, 2, 4, 128), np.float32)
    for mh in range(2):
        for s in range(2):
            wd[:, s, 0 + mh, :] = detw[2 * pp128 + mh][:, 2 * pp128 + s].T
            # rec1B fp8 1-term stationaries (used when b1t)
            wd[:, s, 2 + mh, :] = r1w[mh * 128 + pp128][:, 256 + 2 * pp128 + s].T
    wd8 = wd.astype(ml_dtypes.float8_e4m3)

    # rec2 stationaries with mc folded, per (core, slot)
    w2base = np.zeros((128, 2, LP), np.float32)
    for k in range(2):
        w2base[:, k, 0:L] = r2w[live_ch][:, k * 128 + pp128].T

    in_maps = []
    gather_info = []
    for c in range(NCORES):
        xa = np.zeros((128, 2, STOT), ml_dtypes.bfloat16)
        x8 = np.zeros((128, 2, STOT), ml_dtypes.float8_e4m3)
        LPW = L1 + 128 if L2p else L1
        w2 = np.zeros((128, 2, NSLOT, LPW), np.float32)
        ginfo = []
        for s in range(NSLOT):
            bimg = [bb for bb in range(B)
                    if core_of[bb] == c and slot_of[bb] == s][0]
            idx = np.where(livepx[bimg])[0]
            xa[:, :, OFF[s]:OFF[s] + len(idx)] = xm16[bimg][:, :, idx]
            x8[:, :, OFF[s]:OFF[s] + len(idx)] = xm8[bimg][:, :, idx]
            w2[:, :, s, 0:L] = w2base[:, :, 0:L] * \
                mc[bimg, live_ch][None, None, :].astype(np.float32)
            ginfo.append((bimg, idx, OFF[s]))
        in_maps.append({
            "xall": xa, "x8all": x8,
            "wg": wg16, "wd8": wd8,
            "w2": w2.astype(ml_dtypes.bfloat16),
        })
        gather_info.append(ginfo)
    return in_maps, (live_ch, L1, L2p, gather_info)


def kernel(**inputs) -> np.ndarray:
    from concourse.bass_utils import run_bass_kernel_spmd

    in_maps, (live_ch, L1, L2p, gather_info) = _prep_in_maps(inputs)
    key = (int(L1), int(L2p), _CFG["S"], _CFG["b1t"])
    if _CACHE.get("key") != key:
        _CACHE["nc"] = _build()
        _CACHE["key"] = key
    nc = _CACHE["nc"]
    last_err = None
    for _attempt in range(3):
        try:
            res = run_bass_kernel_spmd(nc, in_maps, core_ids=list(range(NCORES)))
            break
        except Exception as e:
            last_err = e
    else:
        raise last_err
    L = len(live_ch)
    out = np.zeros((B, C, HW), np.float32)
    for c in range(NCORES):
        o2 = np.asarray(res.results[c]["out2"], dtype=np.float32)
        for (bimg, idx, off) in gather_info[c]:
            out[bimg][np.ix_(live_ch, idx)] = o2[0:L, off:off + len(idx)]
    return out.reshape(B, C, H, W)
